# revision 2
# baseline (speedup 1.0000x reference)
"""Trainium2 Bass kernel for DeepOdoModel (CNN feature extractor + GRU).

Strategy (v2):
- Data-parallel over batch: B=16 -> 2 batch elements per core across 8 cores,
  no collectives.
- All folded weights are baked into the NEFF as Const tensors
  (nc.inline_tensor): they are DMA'd to device HBM once at model load, so the
  per-execution input is a single packed tensor per core (raw phone data in
  channel-major layout + transposed h0) -- 1.4 MB instead of ~23 MB across 13
  buffers. kernel() rebuilds (recompiles) only if the weight bytes change.
- conv1 runs as 11 shift-matmuls (K=7) accumulating in PSUM directly from the
  raw channel-major input -- no im2col materialization on host or device.
- conv2 via shift-accumulation (K=128, 9 shifts), fc1+fc2 folded into one
  512x1536 matmul, GRU input projection (gi) precomputed for all timesteps,
  biases all folded host-side at build time.
- GRU: 512 sequential steps; per step 48 matmuls (whh.T stationary tiles in
  bf16 for 2x faster weight loads, h.T moving [128,2]) producing feature-major
  gates, then 8 DVE/ACT ops.
- kernel() keeps a cached jitted 8-core executable; per call it only packs
  and ships the small input, runs, and gathers the [B,T,1] output.
"""

import sys

if "/opt/trn_rl_repo" not in sys.path:
    sys.path.insert(0, "/opt/trn_rl_repo")

import numpy as np

B, T_FULL, L, C = 16, 512, 50, 7
H = 512
NCORES = 8
BL = B // NCORES  # 2
NF_FULL = BL * T_FULL
PHONE_ELEMS = C * NF_FULL * L          # c-major [7, NF, 50]
XIN_ELEMS = PHONE_ELEMS + 128 * 8      # + h0t [128, 8]

WEIGHT_NAMES = (
    "conv1_w", "conv1_b", "conv2_w", "conv2_b", "fc1_w", "fc1_b",
    "fc2_w", "fc2_b", "gru_wih", "gru_whh", "gru_bih", "gru_bhh",
    "fc3_w", "fc3_b",
)


def fold_weights(inputs, whh_bf16=True):
    """Host-side weight folding -> dict of const arrays in SBUF layout."""
    f = np.float32
    conv1_w = np.asarray(inputs["conv1_w"], f)   # [128, 7, 11]
    conv1_b = np.asarray(inputs["conv1_b"], f)
    conv2_w = np.asarray(inputs["conv2_w"], f)   # [256, 128, 9]
    conv2_b = np.asarray(inputs["conv2_b"], f)
    fc1_w = np.asarray(inputs["fc1_w"], f)       # [1024, 1536]
    fc1_b = np.asarray(inputs["fc1_b"], f)
    fc2_w = np.asarray(inputs["fc2_w"], f)       # [512, 1024]
    fc2_b = np.asarray(inputs["fc2_b"], f)
    wih = np.asarray(inputs["gru_wih"], f)       # [1536, 512]
    whh = np.asarray(inputs["gru_whh"], f)
    bih = np.asarray(inputs["gru_bih"], f)
    bhh = np.asarray(inputs["gru_bhh"], f)
    fc3_w = np.asarray(inputs["fc3_w"], f)       # [1, 512]
    fc3_b = np.asarray(inputs["fc3_b"], f)

    # conv1 stationaries: for shift k, lhsT_k = conv1_w[:, :, k].T  [7, 128]
    w1s = np.ascontiguousarray(
        conv1_w.transpose(1, 2, 0).reshape(7, 11 * 128))  # [c, (k, o)]
    w2t = np.ascontiguousarray(
        conv2_w.transpose(1, 2, 0).reshape(128, 9 * 256))  # [i, (k, o)]

    Wc = fc2_w @ fc1_w                      # [512, 1536]
    b2_eff = conv2_b + np.einsum("oik,i->o", conv2_w, conv1_b)
    b2_flat = np.repeat(b2_eff, 6)          # [1536] channel-major flatten
    bc_eff = fc2_w @ fc1_b + fc2_b + Wc @ b2_flat  # [512]

    WcT = Wc.T                              # [1536, 512]
    wct = np.empty((128, 12 * 512), f)      # SBUF layout [128, (kt, m)]
    for p in range(6):
        for cm in range(2):
            kt = p * 2 + cm
            rows = 6 * (cm * 128 + np.arange(128)) + p
            wct[:, kt * 512:(kt + 1) * 512] = WcT[rows]

    gi_bias = bih + wih @ bc_eff
    gi_bias[:1024] += bhh[:1024]            # fold bhh for r,z gates
    gib = np.ascontiguousarray(gi_bias.reshape(12, 128).T)     # [128, 12]
    bhhn = np.ascontiguousarray(bhh[1024:].reshape(4, 128).T)  # [128, 4]
    fc3wt = np.ascontiguousarray(fc3_w[0].reshape(4, 128).T)   # [128, 4]

    wihT = wih.T                            # [512, 1536]
    wihsb = np.empty((128, 4 * 1536), f)
    for k in range(4):
        wihsb[:, k * 1536:(k + 1) * 1536] = wihT[k * 128:(k + 1) * 128]
    whhT = whh.T
    whhsb = np.empty((128, 4 * 1536), f)
    for k in range(4):
        whhsb[:, k * 1536:(k + 1) * 1536] = whhT[k * 128:(k + 1) * 128]
    if whh_bf16:
        import ml_dtypes
        whhsb = whhsb.astype(ml_dtypes.bfloat16)

    return {
        "w1s": w1s, "w2t": w2t, "wct": wct, "wihsb": wihsb,
        "whhsb": whhsb, "gib": gib, "bhhn": bhhn, "fc3w": fc3wt,
        "fc3b": fc3_b.reshape(1, 1).astype(f),
    }


def build_nc(weights, T=T_FULL, num_devices=NCORES, whh_bf16=True):
    import concourse.mybir as mybir
    import concourse.tile as tile
    from concourse import bacc
    from concourse.alu_op_type import AluOpType

    f32 = mybir.dt.float32
    bf16 = mybir.dt.bfloat16
    whh_dt = bf16 if whh_bf16 else f32
    AF = mybir.ActivationFunctionType
    NF = BL * T
    F1 = 8    # conv1 frames/chunk (psum [128, 320])
    F2 = 32   # conv2 frames/block
    F3 = min(128, NF)
    n3 = NF // F3
    n2 = F3 // F2
    n1 = F2 // F1
    phone_elems = C * NF * L

    nc = bacc.Bacc("TRN2", target_bir_lowering=False, debug=False,
                   num_devices=num_devices)

    xin = nc.dram_tensor("xin", [1, phone_elems + 1024], f32,
                         kind="ExternalInput")
    cw1 = nc.inline_tensor(weights["w1s"], name="cw1")
    cw2 = nc.inline_tensor(weights["w2t"], name="cw2")
    cwc = nc.inline_tensor(weights["wct"], name="cwc")
    cwih = nc.inline_tensor(weights["wihsb"], name="cwih")
    cwhh = nc.inline_tensor(weights["whhsb"], name="cwhh")
    cgib = nc.inline_tensor(weights["gib"], name="cgib")
    cbhhn = nc.inline_tensor(weights["bhhn"], name="cbhhn")
    cfc3w = nc.inline_tensor(weights["fc3w"], name="cfc3w")
    cfc3b = nc.inline_tensor(weights["fc3b"], name="cfc3b")
    out = nc.dram_tensor("out", [1, 2 * T], f32, kind="ExternalOutput")

    with tile.TileContext(nc) as tc:
        with tc.tile_pool(name="weights", bufs=1) as wp:
            w1sb = wp.tile([7, 11 * 128], f32)
            nc.sync.dma_start(out=w1sb, in_=cw1.ap())
            w2sb = wp.tile([128, 9 * 256], f32)
            nc.sync.dma_start(out=w2sb, in_=cw2.ap())
            wcsb = wp.tile([128, 12 * 512], f32)
            nc.sync.dma_start(out=wcsb, in_=cwc.ap())
            wihsb = wp.tile([128, 4 * 1536], f32)
            nc.sync.dma_start(out=wihsb, in_=cwih.ap())
            whhsb = wp.tile([128, 4 * 1536], whh_dt)
            nc.sync.dma_start(out=whhsb, in_=cwhh.ap())
            gibsb = wp.tile([128, 12], f32)
            nc.sync.dma_start(out=gibsb, in_=cgib.ap())
            bhhnsb = wp.tile([128, 4], f32)
            nc.sync.dma_start(out=bhhnsb, in_=cbhhn.ap())
            fc3wsb = wp.tile([128, 4], f32)
            nc.sync.dma_start(out=fc3wsb, in_=cfc3w.ap())
            fc3bsb = wp.tile([1, 1], f32)
            nc.sync.dma_start(out=fc3bsb, in_=cfc3b.ap())
            h0sb = wp.tile([128, 8], f32)
            h0view = xin.ap()[0:1, phone_elems:phone_elems + 1024].rearrange(
                "o (p c) -> (o p) c", p=128, c=8)
            nc.sync.dma_start(out=h0sb, in_=h0view)
            if whh_bf16:
                h0sbb = wp.tile([128, 8], bf16)
                nc.vector.tensor_copy(h0sbb, h0sb)

            # persistent activations
            giT = wp.tile([128, 12 * NF], f32)   # (m, b, t) feature-major gi
            hsT = wp.tile([128, T * 8], f32)     # (t, k, b) hidden states
            if whh_bf16:
                hsTb = wp.tile([128, T * 8], bf16)  # bf16 copy for matmul rhs

            # phone DRAM view: [7, NF, 50] channel-major
            pv = xin.ap()[0:1, 0:phone_elems].rearrange(
                "o (c n l) -> (o c) n l", c=C, n=NF, l=L)

            # ---------------- CNN + FC + gi ----------------
            with tc.tile_pool(name="ps_cnn", bufs=6, space="PSUM") as psp, \
                 tc.tile_pool(name="xb", bufs=3) as xpool, \
                 tc.tile_pool(name="p1", bufs=2) as p1pool, \
                 tc.tile_pool(name="p2", bufs=2) as p2pool, \
                 tc.tile_pool(name="ft", bufs=2) as ftpool:
                for b3 in range(n3):
                    p2t = p2pool.tile([128, 2 * F3 * 6], f32)
                    for b2 in range(n2):
                        p1t = p1pool.tile([128, F2 * 20], f32)
                        for c1 in range(n1):
                            n0 = b3 * F3 + b2 * F2 + c1 * F1
                            x1 = xpool.tile([7, F1 * 50], f32)
                            nc.sync.dma_start(
                                out=x1, in_=pv[:, n0:n0 + F1, :])
                            x1v = x1.rearrange("p (n l) -> p n l", l=L)
                            ps1 = psp.tile([128, F1 * 40], f32, tag="ps")
                            for k in range(11):
                                nc.tensor.matmul(
                                    ps1[:],
                                    lhsT=w1sb[:, k * 128:(k + 1) * 128],
                                    rhs=x1v[:, :, k:k + 40],
                                    start=(k == 0), stop=(k == 10),
                                )
                            nc.vector.tensor_reduce(
                                out=p1t[:, c1 * F1 * 20:(c1 + 1) * F1 * 20],
                                in_=ps1.rearrange("p (a two) -> p a two", two=2),
                                axis=mybir.AxisListType.X, op=AluOpType.max,
                            )
                        # conv2 over this 32-frame block
                        p1v = p1t.rearrange("p (n l) -> p n l", l=20)
                        for m in range(2):
                            ps2 = psp.tile([128, F2 * 12], f32, tag="ps")
                            for k in range(9):
                                nc.tensor.matmul(
                                    ps2[:],
                                    lhsT=w2sb[:, k * 256 + m * 128:
                                              k * 256 + m * 128 + 128],
                                    rhs=p1v[:, :, k:k + 12],
                                    start=(k == 0), stop=(k == 8),
                                )
                            nc.vector.tensor_reduce(
                                out=p2t[:, m * F3 * 6 + b2 * F2 * 6:
                                        m * F3 * 6 + (b2 + 1) * F2 * 6],
                                in_=ps2.rearrange("p (a two) -> p a two", two=2),
                                axis=mybir.AxisListType.X, op=AluOpType.max,
                            )
                    # fused fc1*fc2 -> featT
                    ft = ftpool.tile([128, 4 * F3], f32)
                    p2v = p2t.rearrange("p (c n l) -> p c n l", c=2, l=6)
                    for m4 in range(4):
                        ps3 = psp.tile([128, F3], f32, tag="ps")
                        for kt in range(12):
                            p_, cm = kt // 2, kt % 2
                            nc.tensor.matmul(
                                ps3[:],
                                lhsT=wcsb[:, kt * 512 + m4 * 128:
                                          kt * 512 + m4 * 128 + 128],
                                rhs=p2v[:, cm, :, p_:p_ + 1],
                                start=(kt == 0), stop=(kt == 11),
                            )
                        nc.scalar.copy(ft[:, m4 * F3:(m4 + 1) * F3], ps3[:])
                    # gi projection -> giT
                    for m in range(12):
                        ps4 = psp.tile([128, F3], f32, tag="ps")
                        for k in range(4):
                            nc.tensor.matmul(
                                ps4[:],
                                lhsT=wihsb[:, k * 1536 + m * 128:
                                           k * 1536 + m * 128 + 128],
                                rhs=ft[:, k * F3:(k + 1) * F3],
                                start=(k == 0), stop=(k == 3),
                            )
                        nc.scalar.activation(
                            giT[:, m * NF + b3 * F3:m * NF + (b3 + 1) * F3],
                            ps4[:], AF.Identity, bias=gibsb[:, m:m + 1])

            # ---------------- GRU recurrence ----------------
            giv = giT.rearrange("p (m b tt) -> p m b tt", m=12, b=BL)
            MORDER = [0, 1, 2, 3, 8, 9, 10, 11, 4, 5, 6, 7]
            with tc.tile_pool(name="psg", bufs=2, space="PSUM") as psgp, \
                 tc.tile_pool(name="gt", bufs=3) as gtp:
                from concourse.tile_rust import add_dep_helper
                for t in range(T):
                    hprev = h0sb if t == 0 else hsT[:, (t - 1) * 8:t * 8]
                    if whh_bf16:
                        hprev_mm = h0sbb if t == 0 else hsTb[:, (t - 1) * 8:t * 8]
                    else:
                        hprev_mm = hprev
                    psg = psgp.tile([128, 24], f32)
                    # PE instructions must stay in emission order: accumulation
                    # groups share a PSUM bank and start=True clears has_written
                    # bank-wide, so interleaving groups corrupts partial sums.
                    prev_mm = None
                    for m in MORDER:
                        for k in range(4):
                            mm = nc.tensor.matmul(
                                psg[:, 2 * m:2 * m + 2],
                                lhsT=whhsb[:, k * 1536 + m * 128:
                                           k * 1536 + m * 128 + 128],
                                rhs=hprev_mm[:, 2 * k:2 * k + 2],
                                start=(k == 0), stop=(k == 3),
                            )
                            if prev_mm is not None:
                                add_dep_helper(mm.ins, prev_mm.ins,
                                               reason="psum group order")
                            prev_mm = mm
                    # r gate (ready first: m 0-3)
                    rp = gtp.tile([128, 4, 2], f32)
                    nc.vector.tensor_tensor(
                        out=rp,
                        in0=psg[:, 0:8].rearrange("p (m b) -> p m b", b=2),
                        in1=giv[:, 0:4, :, t], op=AluOpType.add)
                    rt = gtp.tile([128, 4, 2], f32)
                    nc.scalar.activation(rt, rp, AF.Sigmoid)
                    # n gate (m 8-11): n = tanh(gi_n + r*(hn + bhh_n))
                    tmp = gtp.tile([128, 4, 2], f32)
                    for j in range(4):
                        nc.vector.scalar_tensor_tensor(
                            out=tmp[:, j:j + 1, :],
                            in0=psg[:, 16 + 2 * j:18 + 2 * j].rearrange(
                                "p (o b) -> p o b", o=1),
                            scalar=bhhnsb[:, j:j + 1],
                            in1=rt[:, j:j + 1, :],
                            op0=AluOpType.add, op1=AluOpType.mult)
                    npre = gtp.tile([128, 4, 2], f32)
                    nc.vector.tensor_tensor(out=npre, in0=tmp,
                                            in1=giv[:, 8:12, :, t],
                                            op=AluOpType.add)
                    nt = gtp.tile([128, 4, 2], f32)
                    nc.scalar.activation(nt, npre, AF.Tanh)
                    hp3 = hprev.rearrange("p (k b) -> p k b", b=2)
                    d = gtp.tile([128, 4, 2], f32)
                    nc.vector.tensor_tensor(out=d, in0=hp3, in1=nt,
                                            op=AluOpType.subtract)
                    # z gate (ready last: m 4-7); h = n + z*(h - n)
                    zp = gtp.tile([128, 4, 2], f32)
                    nc.vector.tensor_tensor(
                        out=zp,
                        in0=psg[:, 8:16].rearrange("p (m b) -> p m b", b=2),
                        in1=giv[:, 4:8, :, t], op=AluOpType.add)
                    zt = gtp.tile([128, 4, 2], f32)
                    nc.scalar.activation(zt, zp, AF.Sigmoid)
                    e = gtp.tile([128, 4, 2], f32)
                    nc.vector.tensor_tensor(out=e, in0=d, in1=zt,
                                            op=AluOpType.mult)
                    hnew = hsT[:, t * 8:(t + 1) * 8].rearrange(
                        "p (k b) -> p k b", b=2)
                    nc.vector.tensor_tensor(out=hnew, in0=e, in1=nt,
                                            op=AluOpType.add)
                    if whh_bf16:
                        nc.vector.tensor_copy(
                            hsTb[:, t * 8:(t + 1) * 8],
                            hsT[:, t * 8:(t + 1) * 8])

            # ---------------- output head ----------------
            with tc.tile_pool(name="pso", bufs=2, space="PSUM") as psop, \
                 tc.tile_pool(name="ot", bufs=1) as otp:
                osb = otp.tile([1, 2 * T], f32)
                hs4 = hsT.rearrange("p (tt k b) -> p tt k b", k=4, b=2)
                tc_chunk = min(256, T)
                for ch in range(T // tc_chunk):
                    pso = psop.tile([1, tc_chunk, 2], f32)
                    for k in range(4):
                        nc.tensor.matmul(
                            pso[:],
                            lhsT=fc3wsb[:, k:k + 1],
                            rhs=hs4[:, ch * tc_chunk:(ch + 1) * tc_chunk, k, :],
                            start=(k == 0), stop=(k == 3),
                        )
                    nc.scalar.activation(
                        osb[:, ch * tc_chunk * 2:(ch + 1) * tc_chunk * 2]
                        .rearrange("p (tt b) -> p tt b", b=2),
                        pso[:], AF.Identity, bias=fc3bsb[:, 0:1])
                nc.sync.dma_start(out=out.ap(), in_=osb)

    nc.compile()
    return nc


def pack_inputs(inputs, T=T_FULL):
    """Per-core packed input: phone c-major [7, NF, 50] + h0t [128, 8]."""
    f = np.float32
    phone = np.asarray(inputs["phone_data"], f)  # [B, T, L, C]
    h0 = np.asarray(inputs["h0"], f)             # [B, H]
    NF = BL * T
    xs = []
    for c in range(NCORES):
        psh = phone[c * BL:(c + 1) * BL].reshape(NF, L, C)
        pcm = np.ascontiguousarray(psh.transpose(2, 0, 1)).reshape(-1)
        h0sh = h0[c * BL:(c + 1) * BL]
        h0tt = np.ascontiguousarray(
            h0sh.reshape(BL, 4, 128).transpose(2, 1, 0)).reshape(-1)
        xs.append(np.concatenate([pcm, h0tt])[None, :])
    return xs  # list of [1, XIN_ELEMS]


def assemble_output(outs, T=T_FULL):
    """outs: list/array of per-core [1, 2T] -> [B, T, 1]."""
    full = np.empty((B, T, 1), np.float32)
    for c in range(NCORES):
        o = np.asarray(outs[c]).reshape(T, BL)  # cols (t, b)
        full[c * BL:(c + 1) * BL, :, 0] = o.T
    return full


class _Runner:
    """Cached jitted 8-core executable for a fixed weight set."""

    def __init__(self, nc):
        import jax
        import concourse.mybir as mybir
        from jax.sharding import Mesh, PartitionSpec
        from jax.experimental.shard_map import shard_map
        from concourse import bass2jax
        from concourse.bass2jax import _bass_exec_p, partition_id_tensor

        bass2jax.install_neuronx_cc_hook()
        self.nc = nc
        partition_name = (nc.partition_id_tensor.name
                          if nc.partition_id_tensor else None)
        in_names, out_names, out_avals, zero_outs = [], [], [], []
        for alloc in nc.m.functions[0].allocations:
            if not isinstance(alloc, mybir.MemoryLocationSet):
                continue
            if alloc.kind == "ExternalInput":
                name = alloc.memorylocations[0].name
                if name != partition_name:
                    in_names.append(name)
            elif alloc.kind == "ExternalOutput":
                shape = tuple(alloc.tensor_shape)
                dtype = mybir.dt.np(alloc.dtype)
                out_names.append(alloc.memorylocations[0].name)
                out_avals.append(jax.core.ShapedArray(shape, dtype))
                zero_outs.append(np.zeros(shape, dtype))
        assert in_names == ["xin"], in_names
        self.out_names = out_names
        self.out_avals = out_avals
        all_in_names = in_names + out_names + (
            [partition_name] if partition_name else [])

        def _body(*args):
            operands = list(args)
            if partition_name is not None:
                operands.append(partition_id_tensor())
            return tuple(_bass_exec_p.bind(
                *operands, out_avals=tuple(out_avals),
                in_names=tuple(all_in_names), out_names=tuple(out_names),
                lowering_input_output_aliases=(), sim_require_finite=True,
                sim_require_nnan=True, nc=nc))

        devices = jax.devices()[:NCORES]
        mesh = Mesh(np.asarray(devices), ("core",))
        nin = len(in_names) + len(out_avals)
        self.sharded = jax.jit(
            shard_map(_body, mesh=mesh,
                      in_specs=(PartitionSpec("core"),) * nin,
                      out_specs=(PartitionSpec("core"),) * len(out_avals),
                      check_rep=False),
            keep_unused=True)
        self.zeros = [jax.device_put(
            np.zeros((NCORES * z.shape[0], *z.shape[1:]), z.dtype))
            for z in zero_outs]
        self.jax = jax

    def put(self, xs):
        return self.jax.device_put(np.concatenate(xs, axis=0))

    def dispatch(self, xin_dev):
        return self.sharded(xin_dev, *self.zeros)

    def run(self, xs):
        outs = self.dispatch(self.put(xs))
        o = np.asarray(outs[0])
        return [o.reshape(NCORES, *self.out_avals[0].shape)[c]
                for c in range(NCORES)]


_CACHE = {}


def _weights_key(inputs):
    import hashlib
    hsh = hashlib.blake2b(digest_size=16)
    for n in WEIGHT_NAMES:
        hsh.update(np.ascontiguousarray(np.asarray(inputs[n])).tobytes())
    return hsh.hexdigest()


def get_runner(inputs):
    key = _weights_key(inputs)
    if _CACHE.get("key") != key:
        weights = fold_weights(inputs)
        nc = build_nc(weights)
        _CACHE.update(key=key, runner=_Runner(nc))
    return _CACHE["runner"]


def kernel(**inputs):
    runner = get_runner(inputs)
    outs = runner.run(pack_inputs(inputs))
    return assemble_output(outs)


# revision 13
# speedup vs baseline: 1.3007x; 1.3007x over previous
"""Trainium2 Bass kernel for DeepOdoModel (CNN feature extractor + GRU).

Strategy (v2):
- Data-parallel over batch: B=16 -> 2 batch elements per core across 8 cores,
  no collectives.
- All folded weights are baked into the NEFF as Const tensors
  (nc.inline_tensor): they are DMA'd to device HBM once at model load, so the
  per-execution input is a single packed tensor per core (raw phone data in
  channel-major layout + transposed h0) -- 1.4 MB instead of ~23 MB across 13
  buffers. kernel() rebuilds (recompiles) only if the weight bytes change.
- conv1 runs as 11 shift-matmuls (K=7) accumulating in PSUM directly from the
  raw channel-major input -- no im2col materialization on host or device.
- conv2 via shift-accumulation (K=128, 9 shifts), fc1+fc2 folded into one
  512x1536 matmul, GRU input projection (gi) precomputed for all timesteps,
  biases all folded host-side at build time.
- GRU: 512 sequential steps; per step 48 matmuls (whh.T stationary tiles in
  bf16 for 2x faster weight loads, h.T moving [128,2]) producing feature-major
  gates, then 8 DVE/ACT ops.
- kernel() keeps a cached jitted 8-core executable; per call it only packs
  and ships the small input, runs, and gathers the [B,T,1] output.
"""

import sys

if "/opt/trn_rl_repo" not in sys.path:
    sys.path.insert(0, "/opt/trn_rl_repo")

import numpy as np

B, T_FULL, L, C = 16, 512, 50, 7
H = 512
NCORES = 8
BL = B // NCORES  # 2
NF_FULL = BL * T_FULL
PHONE_ELEMS = C * NF_FULL * L          # c-major [7, NF, 50], bf16 pairs
XIN_ELEMS = PHONE_ELEMS // 2 + 128 * 8  # f32 words: bf16 phone + f32 h0t

WEIGHT_NAMES = (
    "conv1_w", "conv1_b", "conv2_w", "conv2_b", "fc1_w", "fc1_b",
    "fc2_w", "fc2_b", "gru_wih", "gru_whh", "gru_bih", "gru_bhh",
    "fc3_w", "fc3_b",
)


def fold_weights(inputs, whh_bf16=True):
    """Host-side weight folding -> dict of const arrays in SBUF layout."""
    f = np.float32
    conv1_w = np.asarray(inputs["conv1_w"], f)   # [128, 7, 11]
    conv1_b = np.asarray(inputs["conv1_b"], f)
    conv2_w = np.asarray(inputs["conv2_w"], f)   # [256, 128, 9]
    conv2_b = np.asarray(inputs["conv2_b"], f)
    fc1_w = np.asarray(inputs["fc1_w"], f)       # [1024, 1536]
    fc1_b = np.asarray(inputs["fc1_b"], f)
    fc2_w = np.asarray(inputs["fc2_w"], f)       # [512, 1024]
    fc2_b = np.asarray(inputs["fc2_b"], f)
    wih = np.asarray(inputs["gru_wih"], f)       # [1536, 512]
    whh = np.asarray(inputs["gru_whh"], f)
    bih = np.asarray(inputs["gru_bih"], f)
    bhh = np.asarray(inputs["gru_bhh"], f)
    fc3_w = np.asarray(inputs["fc3_w"], f)       # [1, 512]
    fc3_b = np.asarray(inputs["fc3_b"], f)

    # conv1 stationaries: for shift k, lhsT_k = conv1_w[:, :, k].T  [7, 128]
    w1s = np.ascontiguousarray(
        conv1_w.transpose(1, 2, 0).reshape(7, 11 * 128))  # [c, (k, o)]
    w2t = np.ascontiguousarray(
        conv2_w.transpose(1, 2, 0).reshape(128, 9 * 256))  # [i, (k, o)]

    Wc = fc2_w @ fc1_w                      # [512, 1536]
    b2_eff = conv2_b + np.einsum("oik,i->o", conv2_w, conv1_b)
    b2_flat = np.repeat(b2_eff, 6)          # [1536] channel-major flatten
    bc_eff = fc2_w @ fc1_b + fc2_b + Wc @ b2_flat  # [512]

    WcT = Wc.T                              # [1536, 512]
    wct = np.empty((128, 12 * 512), f)      # SBUF layout [128, (kt, m)]
    for p in range(6):
        for cm in range(2):
            kt = p * 2 + cm
            rows = 6 * (cm * 128 + np.arange(128)) + p
            wct[:, kt * 512:(kt + 1) * 512] = WcT[rows]

    gi_bias = bih + wih @ bc_eff
    gi_bias[:1024] += bhh[:1024]            # fold bhh for r,z gates
    gib = np.ascontiguousarray(gi_bias.reshape(12, 128).T)     # [128, 12]
    bhhn = np.ascontiguousarray(bhh[1024:].reshape(4, 128).T)  # [128, 4]
    fc3wt = np.ascontiguousarray(fc3_w[0].reshape(4, 128).T)   # [128, 4]

    wihT = wih.T                            # [512, 1536]
    wihsb = np.empty((128, 4 * 1536), f)
    for k in range(4):
        wihsb[:, k * 1536:(k + 1) * 1536] = wihT[k * 128:(k + 1) * 128]
    whhT = whh.T
    whhsb = np.empty((128, 4 * 1536), f)
    for k in range(4):
        whhsb[:, k * 1536:(k + 1) * 1536] = whhT[k * 128:(k + 1) * 128]
    import ml_dtypes
    bf = ml_dtypes.bfloat16
    if whh_bf16:
        whhsb = whhsb.astype(bf)

    return {
        "w1s": w1s.astype(bf), "w2t": w2t.astype(bf), "wct": wct.astype(bf),
        "wihsb": wihsb.astype(bf), "whhsb": whhsb, "gib": gib, "bhhn": bhhn,
        "fc3w": fc3wt, "fc3b": fc3_b.reshape(1, 1).astype(f),
    }


def build_nc(weights, T=T_FULL, num_devices=NCORES, whh_bf16=True):
    import concourse.mybir as mybir
    import concourse.tile as tile
    from concourse import bacc
    from concourse.alu_op_type import AluOpType

    f32 = mybir.dt.float32
    bf16 = mybir.dt.bfloat16
    whh_dt = bf16 if whh_bf16 else f32
    AF = mybir.ActivationFunctionType
    NF = BL * T
    F1 = 8    # conv1 frames/chunk (psum [128, 320])
    F2 = 32   # conv2 frames/block
    F3 = min(128, NF)
    n3 = NF // F3
    n2 = F3 // F2
    n1 = F2 // F1
    phone_elems = C * NF * L

    nc = bacc.Bacc("TRN2", target_bir_lowering=False, debug=False,
                   num_devices=num_devices)

    xin = nc.dram_tensor("xin", [1, phone_elems // 2 + 1024], f32,
                         kind="ExternalInput")
    cw1 = nc.inline_tensor(weights["w1s"], name="cw1")
    cw2 = nc.inline_tensor(weights["w2t"], name="cw2")
    cwc = nc.inline_tensor(weights["wct"], name="cwc")
    cwih = nc.inline_tensor(weights["wihsb"], name="cwih")
    cwhh = nc.inline_tensor(weights["whhsb"], name="cwhh")
    cgib = nc.inline_tensor(weights["gib"], name="cgib")
    cbhhn = nc.inline_tensor(weights["bhhn"], name="cbhhn")
    cfc3w = nc.inline_tensor(weights["fc3w"], name="cfc3w")
    cfc3b = nc.inline_tensor(weights["fc3b"], name="cfc3b")
    out = nc.dram_tensor("out", [1, 2 * T], f32, kind="ExternalOutput")

    with tile.TileContext(nc) as tc:
        with tc.tile_pool(name="weights", bufs=1) as wp:
            w1sb = wp.tile([7, 11 * 128], bf16)
            nc.sync.dma_start(out=w1sb, in_=cw1.ap())
            w2sb = wp.tile([128, 9 * 256], bf16)
            nc.sync.dma_start(out=w2sb, in_=cw2.ap())
            wcsb = wp.tile([128, 12 * 512], bf16)
            nc.sync.dma_start(out=wcsb, in_=cwc.ap())
            wihsb = wp.tile([128, 4 * 1536], bf16)
            nc.sync.dma_start(out=wihsb, in_=cwih.ap())
            whhsb = wp.tile([128, 4 * 1536], whh_dt)
            nc.sync.dma_start(out=whhsb, in_=cwhh.ap())
            gibsb = wp.tile([128, 12], f32)
            nc.sync.dma_start(out=gibsb, in_=cgib.ap())
            bhhnsb = wp.tile([128, 4], f32)
            nc.sync.dma_start(out=bhhnsb, in_=cbhhn.ap())
            fc3wsb = wp.tile([128, 4], f32)
            nc.sync.dma_start(out=fc3wsb, in_=cfc3w.ap())
            fc3bsb = wp.tile([1, 1], f32)
            nc.sync.dma_start(out=fc3bsb, in_=cfc3b.ap())
            h0sb = wp.tile([128, 8], f32)
            h0off = phone_elems // 2
            h0view = xin.ap()[0:1, h0off:h0off + 1024].rearrange(
                "o (p c) -> (o p) c", p=128, c=8)
            nc.sync.dma_start(out=h0sb, in_=h0view)
            if whh_bf16:
                h0sbb = wp.tile([128, 8], bf16)
                nc.vector.tensor_copy(h0sbb, h0sb)
            # bhhn broadcast to [128, 4, 2] for single-op n-gate math
            bhhnb3 = wp.tile([128, 4, 2], f32)
            nc.vector.tensor_copy(
                bhhnb3,
                bhhnsb.rearrange("p (f o) -> p f o", o=1).to_broadcast(
                    (128, 4, 2)))

            # persistent activations
            giT = wp.tile([128, 12 * NF], f32)   # (m, b, t) feature-major gi
            hsT = wp.tile([128, T * 8], f32)     # (t, k, b) hidden states
            if whh_bf16:
                hsTb = wp.tile([128, T * 8], bf16)  # bf16 copy for matmul rhs

            # phone DRAM view: [7, NF, 50] channel-major, bf16 packed in f32
            pv = xin.ap()[0:1, 0:phone_elems // 2].bitcast(bf16).rearrange(
                "o (c n l) -> (o c) n l", c=C, n=NF, l=L)

            # ---------------- CNN + FC + gi ----------------
            with tc.tile_pool(name="ps_cnn", bufs=6, space="PSUM") as psp, \
                 tc.tile_pool(name="xb", bufs=3) as xpool, \
                 tc.tile_pool(name="p1", bufs=2) as p1pool, \
                 tc.tile_pool(name="p2", bufs=2) as p2pool, \
                 tc.tile_pool(name="ft", bufs=2) as ftpool:
                for b3 in range(n3):
                    p2t = p2pool.tile([128, 2 * F3 * 6], bf16)
                    for b2 in range(n2):
                        p1t = p1pool.tile([128, F2 * 20], bf16)
                        for c1 in range(n1):
                            n0 = b3 * F3 + b2 * F2 + c1 * F1
                            x1 = xpool.tile([7, F1 * 50], bf16)
                            nc.sync.dma_start(
                                out=x1, in_=pv[:, n0:n0 + F1, :])
                            x1v = x1.rearrange("p (n l) -> p n l", l=L)
                            ps1 = psp.tile([128, F1 * 40], f32, tag="ps")
                            for k in range(11):
                                nc.tensor.matmul(
                                    ps1[:],
                                    lhsT=w1sb[:, k * 128:(k + 1) * 128],
                                    rhs=x1v[:, :, k:k + 40],
                                    start=(k == 0), stop=(k == 10),
                                )
                            nc.vector.tensor_reduce(
                                out=p1t[:, c1 * F1 * 20:(c1 + 1) * F1 * 20],
                                in_=ps1.rearrange("p (a two) -> p a two", two=2),
                                axis=mybir.AxisListType.X, op=AluOpType.max,
                            )
                        # conv2 over this 32-frame block
                        p1v = p1t.rearrange("p (n l) -> p n l", l=20)
                        for m in range(2):
                            ps2 = psp.tile([128, F2 * 12], f32, tag="ps")
                            for k in range(9):
                                nc.tensor.matmul(
                                    ps2[:],
                                    lhsT=w2sb[:, k * 256 + m * 128:
                                              k * 256 + m * 128 + 128],
                                    rhs=p1v[:, :, k:k + 12],
                                    start=(k == 0), stop=(k == 8),
                                )
                            nc.vector.tensor_reduce(
                                out=p2t[:, m * F3 * 6 + b2 * F2 * 6:
                                        m * F3 * 6 + (b2 + 1) * F2 * 6],
                                in_=ps2.rearrange("p (a two) -> p a two", two=2),
                                axis=mybir.AxisListType.X, op=AluOpType.max,
                            )
                    # fused fc1*fc2 -> featT
                    ft = ftpool.tile([128, 4 * F3], bf16)
                    p2v = p2t.rearrange("p (c n l) -> p c n l", c=2, l=6)
                    for m4 in range(4):
                        ps3 = psp.tile([128, F3], f32, tag="ps")
                        for kt in range(12):
                            p_, cm = kt // 2, kt % 2
                            nc.tensor.matmul(
                                ps3[:],
                                lhsT=wcsb[:, kt * 512 + m4 * 128:
                                          kt * 512 + m4 * 128 + 128],
                                rhs=p2v[:, cm, :, p_:p_ + 1],
                                start=(kt == 0), stop=(kt == 11),
                            )
                        nc.scalar.copy(ft[:, m4 * F3:(m4 + 1) * F3], ps3[:])
                    # gi projection -> giT
                    for m in range(12):
                        ps4 = psp.tile([128, F3], f32, tag="ps")
                        for k in range(4):
                            nc.tensor.matmul(
                                ps4[:],
                                lhsT=wihsb[:, k * 1536 + m * 128:
                                           k * 1536 + m * 128 + 128],
                                rhs=ft[:, k * F3:(k + 1) * F3],
                                start=(k == 0), stop=(k == 3),
                            )
                        nc.scalar.activation(
                            giT[:, m * NF + b3 * F3:m * NF + (b3 + 1) * F3],
                            ps4[:], AF.Identity, bias=gibsb[:, m:m + 1])

            # ---------------- GRU recurrence ----------------
            giv = giT.rearrange("p (m b tt) -> p m b tt", m=12, b=BL)
            MORDER = [0, 1, 2, 3, 8, 9, 10, 11, 4, 5, 6, 7]
            with tc.tile_pool(name="psg", bufs=2, space="PSUM") as psgp, \
                 tc.tile_pool(name="gt", bufs=3) as gtp:
                from concourse.tile_rust import add_dep_helper
                for t in range(T):
                    hprev = h0sb if t == 0 else hsT[:, (t - 1) * 8:t * 8]
                    if whh_bf16:
                        hprev_mm = h0sbb if t == 0 else hsTb[:, (t - 1) * 8:t * 8]
                    else:
                        hprev_mm = hprev
                    psg = psgp.tile([128, 24], f32)
                    # PE instructions must stay in emission order: accumulation
                    # groups share a PSUM bank and start=True clears has_written
                    # bank-wide, so interleaving groups corrupts partial sums.
                    prev_mm = None
                    for m in MORDER:
                        for k in range(4):
                            mm = nc.tensor.matmul(
                                psg[:, 2 * m:2 * m + 2],
                                lhsT=whhsb[:, k * 1536 + m * 128:
                                           k * 1536 + m * 128 + 128],
                                rhs=hprev_mm[:, 2 * k:2 * k + 2],
                                start=(k == 0), stop=(k == 3),
                            )
                            if prev_mm is not None:
                                add_dep_helper(mm.ins, prev_mm.ins,
                                               reason="psum group order")
                            prev_mm = mm
                    # r gate (ready first: m 0-3)
                    rp = gtp.tile([128, 4, 2], f32)
                    nc.vector.tensor_tensor(
                        out=rp,
                        in0=psg[:, 0:8].rearrange("p (m b) -> p m b", b=2),
                        in1=giv[:, 0:4, :, t], op=AluOpType.add)
                    rt = gtp.tile([128, 4, 2], f32)
                    nc.scalar.activation(rt, rp, AF.Sigmoid)
                    # n gate (m 8-11): n = tanh(gi_n + r*(hn + bhh_n))
                    tmp = gtp.tile([128, 4, 2], f32)
                    nc.vector.tensor_tensor(
                        out=tmp,
                        in0=psg[:, 16:24].rearrange("p (m b) -> p m b", b=2),
                        in1=bhhnb3, op=AluOpType.add)
                    tmp2 = gtp.tile([128, 4, 2], f32)
                    nc.vector.tensor_tensor(out=tmp2, in0=tmp, in1=rt,
                                            op=AluOpType.mult)
                    npre = gtp.tile([128, 4, 2], f32)
                    nc.vector.tensor_tensor(out=npre, in0=tmp2,
                                            in1=giv[:, 8:12, :, t],
                                            op=AluOpType.add)
                    nt = gtp.tile([128, 4, 2], f32)
                    nc.scalar.activation(nt, npre, AF.Tanh)
                    hp3 = hprev.rearrange("p (k b) -> p k b", b=2)
                    d = gtp.tile([128, 4, 2], f32)
                    nc.vector.tensor_tensor(out=d, in0=hp3, in1=nt,
                                            op=AluOpType.subtract)
                    # z gate (ready last: m 4-7); h = n + z*(h - n)
                    zp = gtp.tile([128, 4, 2], f32)
                    nc.vector.tensor_tensor(
                        out=zp,
                        in0=psg[:, 8:16].rearrange("p (m b) -> p m b", b=2),
                        in1=giv[:, 4:8, :, t], op=AluOpType.add)
                    zt = gtp.tile([128, 4, 2], f32)
                    nc.scalar.activation(zt, zp, AF.Sigmoid)
                    e = gtp.tile([128, 4, 2], f32)
                    nc.vector.tensor_tensor(out=e, in0=d, in1=zt,
                                            op=AluOpType.mult)
                    hnew = hsT[:, t * 8:(t + 1) * 8].rearrange(
                        "p (k b) -> p k b", b=2)
                    nc.vector.tensor_tensor(out=hnew, in0=e, in1=nt,
                                            op=AluOpType.add)
                    if whh_bf16:
                        nc.vector.tensor_copy(
                            hsTb[:, t * 8:(t + 1) * 8],
                            hsT[:, t * 8:(t + 1) * 8])

            # ---------------- output head ----------------
            with tc.tile_pool(name="pso", bufs=2, space="PSUM") as psop, \
                 tc.tile_pool(name="ot", bufs=1) as otp:
                osb = otp.tile([1, 2 * T], f32)
                hs4 = hsT.rearrange("p (tt k b) -> p tt k b", k=4, b=2)
                tc_chunk = min(256, T)
                for ch in range(T // tc_chunk):
                    pso = psop.tile([1, tc_chunk, 2], f32)
                    for k in range(4):
                        nc.tensor.matmul(
                            pso[:],
                            lhsT=fc3wsb[:, k:k + 1],
                            rhs=hs4[:, ch * tc_chunk:(ch + 1) * tc_chunk, k, :],
                            start=(k == 0), stop=(k == 3),
                        )
                    nc.scalar.activation(
                        osb[:, ch * tc_chunk * 2:(ch + 1) * tc_chunk * 2]
                        .rearrange("p (tt b) -> p tt b", b=2),
                        pso[:], AF.Identity, bias=fc3bsb[:, 0:1])
                nc.sync.dma_start(out=out.ap(), in_=osb)

    nc.compile()
    return nc


def pack_inputs(inputs, T=T_FULL):
    """Per-core packed input: bf16 phone c-major [7, NF, 50] (bit-packed in
    f32 words) + f32 h0t [128, 8]."""
    import ml_dtypes
    f = np.float32
    phone = np.asarray(inputs["phone_data"], f)  # [B, T, L, C]
    h0 = np.asarray(inputs["h0"], f)             # [B, H]
    NF = BL * T
    xs = []
    for c in range(NCORES):
        psh = phone[c * BL:(c + 1) * BL].reshape(NF, L, C)
        pcm = np.ascontiguousarray(psh.transpose(2, 0, 1)).reshape(-1)
        pb = pcm.astype(ml_dtypes.bfloat16).view(np.uint16).view(f)
        h0sh = h0[c * BL:(c + 1) * BL]
        h0tt = np.ascontiguousarray(
            h0sh.reshape(BL, 4, 128).transpose(2, 1, 0)).reshape(-1)
        xs.append(np.concatenate([pb, h0tt])[None, :])
    return xs  # list of [1, XIN_ELEMS]


def assemble_output(outs, T=T_FULL):
    """outs: list/array of per-core [1, 2T] -> [B, T, 1]."""
    full = np.empty((B, T, 1), np.float32)
    for c in range(NCORES):
        o = np.asarray(outs[c]).reshape(T, BL)  # cols (t, b)
        full[c * BL:(c + 1) * BL, :, 0] = o.T
    return full


class _Runner:
    """Cached jitted 8-core executable for a fixed weight set."""

    def __init__(self, nc):
        import jax
        import concourse.mybir as mybir
        from jax.sharding import Mesh, PartitionSpec
        from jax.experimental.shard_map import shard_map
        from concourse import bass2jax
        from concourse.bass2jax import _bass_exec_p, partition_id_tensor

        bass2jax.install_neuronx_cc_hook()
        self.nc = nc
        partition_name = (nc.partition_id_tensor.name
                          if nc.partition_id_tensor else None)
        in_names, out_names, out_avals, zero_outs = [], [], [], []
        for alloc in nc.m.functions[0].allocations:
            if not isinstance(alloc, mybir.MemoryLocationSet):
                continue
            if alloc.kind == "ExternalInput":
                name = alloc.memorylocations[0].name
                if name != partition_name:
                    in_names.append(name)
            elif alloc.kind == "ExternalOutput":
                shape = tuple(alloc.tensor_shape)
                dtype = mybir.dt.np(alloc.dtype)
                out_names.append(alloc.memorylocations[0].name)
                out_avals.append(jax.core.ShapedArray(shape, dtype))
                zero_outs.append(np.zeros(shape, dtype))
        assert in_names == ["xin"], in_names
        self.out_names = out_names
        self.out_avals = out_avals
        all_in_names = in_names + out_names + (
            [partition_name] if partition_name else [])

        def _body(*args):
            operands = list(args)
            if partition_name is not None:
                operands.append(partition_id_tensor())
            return tuple(_bass_exec_p.bind(
                *operands, out_avals=tuple(out_avals),
                in_names=tuple(all_in_names), out_names=tuple(out_names),
                lowering_input_output_aliases=(), sim_require_finite=True,
                sim_require_nnan=True, nc=nc))

        devices = jax.devices()[:NCORES]
        mesh = Mesh(np.asarray(devices), ("core",))
        nin = len(in_names) + len(out_avals)
        self.sharded = jax.jit(
            shard_map(_body, mesh=mesh,
                      in_specs=(PartitionSpec("core"),) * nin,
                      out_specs=(PartitionSpec("core"),) * len(out_avals),
                      check_rep=False),
            keep_unused=True)
        self.zeros = [jax.device_put(
            np.zeros((NCORES * z.shape[0], *z.shape[1:]), z.dtype))
            for z in zero_outs]
        self.jax = jax

    def put(self, xs):
        return self.jax.device_put(np.concatenate(xs, axis=0))

    def dispatch(self, xin_dev):
        return self.sharded(xin_dev, *self.zeros)

    def run(self, xs):
        outs = self.dispatch(self.put(xs))
        o = np.asarray(outs[0])
        return [o.reshape(NCORES, *self.out_avals[0].shape)[c]
                for c in range(NCORES)]


_CACHE = {}


def _weights_key(inputs):
    import hashlib
    hsh = hashlib.blake2b(digest_size=16)
    for n in WEIGHT_NAMES:
        hsh.update(np.ascontiguousarray(np.asarray(inputs[n])).tobytes())
    return hsh.hexdigest()


def get_runner(inputs):
    key = _weights_key(inputs)
    if _CACHE.get("key") != key:
        weights = fold_weights(inputs)
        nc = build_nc(weights)
        _CACHE.update(key=key, runner=_Runner(nc))
    return _CACHE["runner"]


def kernel(**inputs):
    runner = get_runner(inputs)
    outs = runner.run(pack_inputs(inputs))
    return assemble_output(outs)


# revision 14
# speedup vs baseline: 1.7419x; 1.3392x over previous
"""Trainium2 Bass kernel for DeepOdoModel (CNN feature extractor + GRU).

Strategy (v2):
- Data-parallel over batch: B=16 -> 2 batch elements per core across 8 cores,
  no collectives.
- All folded weights are baked into the NEFF as Const tensors
  (nc.inline_tensor): they are DMA'd to device HBM once at model load, so the
  per-execution input is a single packed tensor per core (raw phone data in
  channel-major layout + transposed h0) -- 1.4 MB instead of ~23 MB across 13
  buffers. kernel() rebuilds (recompiles) only if the weight bytes change.
- conv1 runs as 11 shift-matmuls (K=7) accumulating in PSUM directly from the
  raw channel-major input -- no im2col materialization on host or device.
- conv2 via shift-accumulation (K=128, 9 shifts), fc1+fc2 folded into one
  512x1536 matmul, GRU input projection (gi) precomputed for all timesteps,
  biases all folded host-side at build time.
- GRU: 512 sequential steps; per step 48 matmuls (whh.T stationary tiles in
  bf16 for 2x faster weight loads, h.T moving [128,2]) producing feature-major
  gates, then 8 DVE/ACT ops.
- kernel() keeps a cached jitted 8-core executable; per call it only packs
  and ships the small input, runs, and gathers the [B,T,1] output.
"""

import sys

if "/opt/trn_rl_repo" not in sys.path:
    sys.path.insert(0, "/opt/trn_rl_repo")

import numpy as np

B, T_FULL, L, C = 16, 512, 50, 7
H = 512
NCORES = 8
BL = B // NCORES  # 2
NF_FULL = BL * T_FULL
PHONE_ELEMS = C * NF_FULL * L          # c-major [7, NF, 50], bf16 pairs
XIN_ELEMS = PHONE_ELEMS // 2 + 128 * 8  # f32 words: bf16 phone + f32 h0t

WEIGHT_NAMES = (
    "conv1_w", "conv1_b", "conv2_w", "conv2_b", "fc1_w", "fc1_b",
    "fc2_w", "fc2_b", "gru_wih", "gru_whh", "gru_bih", "gru_bhh",
    "fc3_w", "fc3_b",
)


def fold_weights(inputs, whh_bf16=True):
    """Host-side weight folding -> dict of const arrays in SBUF layout."""
    f = np.float32
    conv1_w = np.asarray(inputs["conv1_w"], f)   # [128, 7, 11]
    conv1_b = np.asarray(inputs["conv1_b"], f)
    conv2_w = np.asarray(inputs["conv2_w"], f)   # [256, 128, 9]
    conv2_b = np.asarray(inputs["conv2_b"], f)
    fc1_w = np.asarray(inputs["fc1_w"], f)       # [1024, 1536]
    fc1_b = np.asarray(inputs["fc1_b"], f)
    fc2_w = np.asarray(inputs["fc2_w"], f)       # [512, 1024]
    fc2_b = np.asarray(inputs["fc2_b"], f)
    wih = np.asarray(inputs["gru_wih"], f)       # [1536, 512]
    whh = np.asarray(inputs["gru_whh"], f)
    bih = np.asarray(inputs["gru_bih"], f)
    bhh = np.asarray(inputs["gru_bhh"], f)
    fc3_w = np.asarray(inputs["fc3_w"], f)       # [1, 512]
    fc3_b = np.asarray(inputs["fc3_b"], f)

    # conv1 stationaries: for shift k, lhsT_k = conv1_w[:, :, k].T  [7, 128]
    w1s = np.ascontiguousarray(
        conv1_w.transpose(1, 2, 0).reshape(7, 11 * 128))  # [c, (k, o)]
    w2t = np.ascontiguousarray(
        conv2_w.transpose(1, 2, 0).reshape(128, 9 * 256))  # [i, (k, o)]

    Wc = fc2_w @ fc1_w                      # [512, 1536]
    b2_eff = conv2_b + np.einsum("oik,i->o", conv2_w, conv1_b)
    b2_flat = np.repeat(b2_eff, 6)          # [1536] channel-major flatten
    bc_eff = fc2_w @ fc1_b + fc2_b + Wc @ b2_flat  # [512]

    WcT = Wc.T                              # [1536, 512]
    wct = np.empty((128, 12 * 512), f)      # SBUF layout [128, (kt, m)]
    for p in range(6):
        for cm in range(2):
            kt = p * 2 + cm
            rows = 6 * (cm * 128 + np.arange(128)) + p
            wct[:, kt * 512:(kt + 1) * 512] = WcT[rows]

    gi_bias = bih + wih @ bc_eff
    gi_bias[:1024] += bhh[:1024]            # fold bhh for r,z gates
    gib = np.ascontiguousarray(gi_bias.reshape(12, 128).T)     # [128, 12]
    bhhn = np.ascontiguousarray(bhh[1024:].reshape(4, 128).T)  # [128, 4]
    fc3wt = np.ascontiguousarray(fc3_w[0].reshape(4, 128).T)   # [128, 4]

    wihT = wih.T                            # [512, 1536]
    wihsb = np.empty((128, 4 * 1536), f)
    for k in range(4):
        wihsb[:, k * 1536:(k + 1) * 1536] = wihT[k * 128:(k + 1) * 128]
    whhT = whh.T
    whhsb = np.empty((128, 4 * 1536), f)
    for k in range(4):
        whhsb[:, k * 1536:(k + 1) * 1536] = whhT[k * 128:(k + 1) * 128]
    import ml_dtypes
    bf = ml_dtypes.bfloat16
    if whh_bf16:
        whhsb = whhsb.astype(bf)

    return {
        "w1s": w1s.astype(bf), "w2t": w2t.astype(bf), "wct": wct.astype(bf),
        "wihsb": wihsb.astype(bf), "whhsb": whhsb, "gib": gib, "bhhn": bhhn,
        "fc3w": fc3wt, "fc3b": fc3_b.reshape(1, 1).astype(f),
    }


def build_nc(weights, T=T_FULL, num_devices=NCORES, whh_bf16=True):
    import concourse.mybir as mybir
    import concourse.tile as tile
    from concourse import bacc
    from concourse.alu_op_type import AluOpType

    f32 = mybir.dt.float32
    bf16 = mybir.dt.bfloat16
    whh_dt = bf16 if whh_bf16 else f32
    AF = mybir.ActivationFunctionType
    NF = BL * T
    F1 = 8    # conv1 frames/chunk (psum [128, 320])
    F2 = 32   # conv2 frames/block
    F3 = min(128, NF)
    n3 = NF // F3
    n2 = F3 // F2
    n1 = F2 // F1
    phone_elems = C * NF * L

    nc = bacc.Bacc("TRN2", target_bir_lowering=False, debug=False,
                   num_devices=num_devices)

    xin = nc.dram_tensor("xin", [1, phone_elems // 2 + 1024], f32,
                         kind="ExternalInput")
    cw1 = nc.inline_tensor(weights["w1s"], name="cw1")
    cw2 = nc.inline_tensor(weights["w2t"], name="cw2")
    cwc = nc.inline_tensor(weights["wct"], name="cwc")
    cwih = nc.inline_tensor(weights["wihsb"], name="cwih")
    cwhh = nc.inline_tensor(weights["whhsb"], name="cwhh")
    cgib = nc.inline_tensor(weights["gib"], name="cgib")
    cbhhn = nc.inline_tensor(weights["bhhn"], name="cbhhn")
    cfc3w = nc.inline_tensor(weights["fc3w"], name="cfc3w")
    cfc3b = nc.inline_tensor(weights["fc3b"], name="cfc3b")
    out = nc.dram_tensor("out", [1, 2 * T], f32, kind="ExternalOutput")

    with tile.TileContext(nc) as tc:
        with tc.tile_pool(name="weights", bufs=1) as wp:
            w1sb = wp.tile([7, 11 * 128], bf16)
            nc.sync.dma_start(out=w1sb, in_=cw1.ap())
            w2sb = wp.tile([128, 9 * 256], bf16)
            nc.sync.dma_start(out=w2sb, in_=cw2.ap())
            wcsb = wp.tile([128, 12 * 512], bf16)
            nc.sync.dma_start(out=wcsb, in_=cwc.ap())
            wihsb = wp.tile([128, 4 * 1536], bf16)
            nc.sync.dma_start(out=wihsb, in_=cwih.ap())
            whhsb = wp.tile([128, 4 * 1536], whh_dt)
            nc.sync.dma_start(out=whhsb, in_=cwhh.ap())
            gibsb = wp.tile([128, 12], f32)
            nc.sync.dma_start(out=gibsb, in_=cgib.ap())
            bhhnsb = wp.tile([128, 4], f32)
            nc.sync.dma_start(out=bhhnsb, in_=cbhhn.ap())
            fc3wsb = wp.tile([128, 4], f32)
            nc.sync.dma_start(out=fc3wsb, in_=cfc3w.ap())
            fc3bsb = wp.tile([1, 1], f32)
            nc.sync.dma_start(out=fc3bsb, in_=cfc3b.ap())
            h0sb = wp.tile([128, 8], f32)
            h0off = phone_elems // 2
            h0view = xin.ap()[0:1, h0off:h0off + 1024].rearrange(
                "o (p c) -> (o p) c", p=128, c=8)
            nc.sync.dma_start(out=h0sb, in_=h0view)
            if whh_bf16:
                h0sbb = wp.tile([128, 8], bf16)
                nc.vector.tensor_copy(h0sbb, h0sb)
            # bhhn broadcast to [128, 4, 2] for single-op n-gate math
            bhhnb3 = wp.tile([128, 4, 2], f32)
            nc.vector.tensor_copy(
                bhhnb3,
                bhhnsb.rearrange("p (f o) -> p f o", o=1).to_broadcast(
                    (128, 4, 2)))

            # persistent activations
            giT = wp.tile([128, 12 * NF], f32)   # (m, b, t) feature-major gi
            hsT = wp.tile([128, T * 8], f32)     # (t, k, b) hidden states
            if whh_bf16:
                hsTb = wp.tile([128, T * 8], bf16)  # bf16 copy for matmul rhs

            # phone DRAM view: [7, NF, 50] channel-major, bf16 packed in f32
            pv = xin.ap()[0:1, 0:phone_elems // 2].bitcast(bf16).rearrange(
                "o (c n l) -> (o c) n l", c=C, n=NF, l=L)

            # ---------------- CNN + FC + gi ----------------
            with tc.tile_pool(name="ps_cnn", bufs=6, space="PSUM") as psp, \
                 tc.tile_pool(name="xb", bufs=3) as xpool, \
                 tc.tile_pool(name="p1", bufs=2) as p1pool, \
                 tc.tile_pool(name="p2", bufs=2) as p2pool, \
                 tc.tile_pool(name="ft", bufs=2) as ftpool:
                for b3 in range(n3):
                    p2t = p2pool.tile([128, 2 * F3 * 6], bf16)
                    for b2 in range(n2):
                        p1t = p1pool.tile([128, F2 * 20], bf16)
                        for c1 in range(n1):
                            n0 = b3 * F3 + b2 * F2 + c1 * F1
                            x1 = xpool.tile([7, F1 * 50], bf16)
                            nc.sync.dma_start(
                                out=x1, in_=pv[:, n0:n0 + F1, :])
                            x1v = x1.rearrange("p (n l) -> p n l", l=L)
                            ps1 = psp.tile([128, F1 * 40], f32, tag="ps")
                            for k in range(11):
                                nc.tensor.matmul(
                                    ps1[:],
                                    lhsT=w1sb[:, k * 128:(k + 1) * 128],
                                    rhs=x1v[:, :, k:k + 40],
                                    start=(k == 0), stop=(k == 10),
                                )
                            nc.vector.tensor_reduce(
                                out=p1t[:, c1 * F1 * 20:(c1 + 1) * F1 * 20],
                                in_=ps1.rearrange("p (a two) -> p a two", two=2),
                                axis=mybir.AxisListType.X, op=AluOpType.max,
                            )
                        # conv2 over this 32-frame block
                        p1v = p1t.rearrange("p (n l) -> p n l", l=20)
                        for m in range(2):
                            ps2 = psp.tile([128, F2 * 12], f32, tag="ps")
                            for k in range(9):
                                nc.tensor.matmul(
                                    ps2[:],
                                    lhsT=w2sb[:, k * 256 + m * 128:
                                              k * 256 + m * 128 + 128],
                                    rhs=p1v[:, :, k:k + 12],
                                    start=(k == 0), stop=(k == 8),
                                )
                            nc.vector.tensor_reduce(
                                out=p2t[:, m * F3 * 6 + b2 * F2 * 6:
                                        m * F3 * 6 + (b2 + 1) * F2 * 6],
                                in_=ps2.rearrange("p (a two) -> p a two", two=2),
                                axis=mybir.AxisListType.X, op=AluOpType.max,
                            )
                    # fused fc1*fc2 -> featT
                    ft = ftpool.tile([128, 4 * F3], bf16)
                    p2v = p2t.rearrange("p (c n l) -> p c n l", c=2, l=6)
                    for m4 in range(4):
                        ps3 = psp.tile([128, F3], f32, tag="ps")
                        for kt in range(12):
                            p_, cm = kt // 2, kt % 2
                            nc.tensor.matmul(
                                ps3[:],
                                lhsT=wcsb[:, kt * 512 + m4 * 128:
                                          kt * 512 + m4 * 128 + 128],
                                rhs=p2v[:, cm, :, p_:p_ + 1],
                                start=(kt == 0), stop=(kt == 11),
                            )
                        nc.scalar.copy(ft[:, m4 * F3:(m4 + 1) * F3], ps3[:])
                    # gi projection -> giT
                    for m in range(12):
                        ps4 = psp.tile([128, F3], f32, tag="ps")
                        for k in range(4):
                            nc.tensor.matmul(
                                ps4[:],
                                lhsT=wihsb[:, k * 1536 + m * 128:
                                           k * 1536 + m * 128 + 128],
                                rhs=ft[:, k * F3:(k + 1) * F3],
                                start=(k == 0), stop=(k == 3),
                            )
                        nc.scalar.activation(
                            giT[:, m * NF + b3 * F3:m * NF + (b3 + 1) * F3],
                            ps4[:], AF.Identity, bias=gibsb[:, m:m + 1])

            # ---------------- GRU recurrence ----------------
            giv = giT.rearrange("p (m b tt) -> p m b tt", m=12, b=BL)
            MORDER = [0, 1, 2, 3, 8, 9, 10, 11, 4, 5, 6, 7]
            with tc.tile_pool(name="psg", bufs=2, space="PSUM") as psgp, \
                 tc.tile_pool(name="gt", bufs=3) as gtp:
                from concourse.tile_rust import add_dep_helper
                for t in range(T):
                    hprev = h0sb if t == 0 else hsT[:, (t - 1) * 8:t * 8]
                    if whh_bf16:
                        hprev_mm = h0sbb if t == 0 else hsTb[:, (t - 1) * 8:t * 8]
                    else:
                        hprev_mm = hprev
                    psg = psgp.tile([128, 24], f32)
                    # PE instructions must stay in emission order: accumulation
                    # groups share a PSUM bank and start=True clears has_written
                    # bank-wide, so interleaving groups corrupts partial sums.
                    prev_mm = None
                    for m in MORDER:
                        for k in range(4):
                            mm = nc.tensor.matmul(
                                psg[:, 2 * m:2 * m + 2],
                                lhsT=whhsb[:, k * 1536 + m * 128:
                                           k * 1536 + m * 128 + 128],
                                rhs=hprev_mm[:, 2 * k:2 * k + 2],
                                start=(k == 0), stop=(k == 3),
                            )
                            if prev_mm is not None:
                                add_dep_helper(mm.ins, prev_mm.ins,
                                               reason="psum group order")
                            prev_mm = mm
                    # r gate (ready first: m 0-3)
                    rp = gtp.tile([128, 4, 2], f32)
                    nc.vector.tensor_tensor(
                        out=rp,
                        in0=psg[:, 0:8].rearrange("p (m b) -> p m b", b=2),
                        in1=giv[:, 0:4, :, t], op=AluOpType.add)
                    rt = gtp.tile([128, 4, 2], f32)
                    nc.scalar.activation(rt, rp, AF.Sigmoid)
                    # n gate (m 8-11): n = tanh(gi_n + r*(hn + bhh_n))
                    tmp = gtp.tile([128, 4, 2], f32)
                    nc.vector.tensor_tensor(
                        out=tmp,
                        in0=psg[:, 16:24].rearrange("p (m b) -> p m b", b=2),
                        in1=bhhnb3, op=AluOpType.add)
                    tmp2 = gtp.tile([128, 4, 2], f32)
                    nc.vector.tensor_tensor(out=tmp2, in0=tmp, in1=rt,
                                            op=AluOpType.mult)
                    npre = gtp.tile([128, 4, 2], f32)
                    nc.vector.tensor_tensor(out=npre, in0=tmp2,
                                            in1=giv[:, 8:12, :, t],
                                            op=AluOpType.add)
                    nt = gtp.tile([128, 4, 2], f32)
                    nc.scalar.activation(nt, npre, AF.Tanh)
                    hp3 = hprev.rearrange("p (k b) -> p k b", b=2)
                    d = gtp.tile([128, 4, 2], f32)
                    nc.vector.tensor_tensor(out=d, in0=hp3, in1=nt,
                                            op=AluOpType.subtract)
                    # z gate (ready last: m 4-7); h = n + z*(h - n)
                    zp = gtp.tile([128, 4, 2], f32)
                    nc.vector.tensor_tensor(
                        out=zp,
                        in0=psg[:, 8:16].rearrange("p (m b) -> p m b", b=2),
                        in1=giv[:, 4:8, :, t], op=AluOpType.add)
                    zt = gtp.tile([128, 4, 2], f32)
                    nc.scalar.activation(zt, zp, AF.Sigmoid)
                    e = gtp.tile([128, 4, 2], f32)
                    nc.vector.tensor_tensor(out=e, in0=d, in1=zt,
                                            op=AluOpType.mult)
                    hnew = hsT[:, t * 8:(t + 1) * 8].rearrange(
                        "p (k b) -> p k b", b=2)
                    if whh_bf16:
                        # bf16 copy of h feeds the next step's matmuls: write
                        # it FIRST (independent e+nt add, not a cast of hsT)
                        # so the recurrence unblocks one DVE op earlier; the
                        # f32 write overlaps the next step's matmul stream.
                        hnewb = hsTb[:, t * 8:(t + 1) * 8].rearrange(
                            "p (k b) -> p k b", b=2)
                        nc.vector.tensor_tensor(out=hnewb, in0=e, in1=nt,
                                                op=AluOpType.add)
                    nc.vector.tensor_tensor(out=hnew, in0=e, in1=nt,
                                            op=AluOpType.add)

            # ---------------- output head ----------------
            with tc.tile_pool(name="pso", bufs=2, space="PSUM") as psop, \
                 tc.tile_pool(name="ot", bufs=1) as otp:
                osb = otp.tile([1, 2 * T], f32)
                hs4 = hsT.rearrange("p (tt k b) -> p tt k b", k=4, b=2)
                tc_chunk = min(256, T)
                for ch in range(T // tc_chunk):
                    pso = psop.tile([1, tc_chunk, 2], f32)
                    for k in range(4):
                        nc.tensor.matmul(
                            pso[:],
                            lhsT=fc3wsb[:, k:k + 1],
                            rhs=hs4[:, ch * tc_chunk:(ch + 1) * tc_chunk, k, :],
                            start=(k == 0), stop=(k == 3),
                        )
                    nc.scalar.activation(
                        osb[:, ch * tc_chunk * 2:(ch + 1) * tc_chunk * 2]
                        .rearrange("p (tt b) -> p tt b", b=2),
                        pso[:], AF.Identity, bias=fc3bsb[:, 0:1])
                nc.sync.dma_start(out=out.ap(), in_=osb)

    nc.compile()
    return nc


def pack_inputs(inputs, T=T_FULL):
    """Per-core packed input: bf16 phone c-major [7, NF, 50] (bit-packed in
    f32 words) + f32 h0t [128, 8]."""
    import ml_dtypes
    f = np.float32
    phone = np.asarray(inputs["phone_data"], f)  # [B, T, L, C]
    h0 = np.asarray(inputs["h0"], f)             # [B, H]
    NF = BL * T
    xs = []
    for c in range(NCORES):
        psh = phone[c * BL:(c + 1) * BL].reshape(NF, L, C)
        pcm = np.ascontiguousarray(psh.transpose(2, 0, 1)).reshape(-1)
        pb = pcm.astype(ml_dtypes.bfloat16).view(np.uint16).view(f)
        h0sh = h0[c * BL:(c + 1) * BL]
        h0tt = np.ascontiguousarray(
            h0sh.reshape(BL, 4, 128).transpose(2, 1, 0)).reshape(-1)
        xs.append(np.concatenate([pb, h0tt])[None, :])
    return xs  # list of [1, XIN_ELEMS]


def assemble_output(outs, T=T_FULL):
    """outs: list/array of per-core [1, 2T] -> [B, T, 1]."""
    full = np.empty((B, T, 1), np.float32)
    for c in range(NCORES):
        o = np.asarray(outs[c]).reshape(T, BL)  # cols (t, b)
        full[c * BL:(c + 1) * BL, :, 0] = o.T
    return full


class _Runner:
    """Cached jitted 8-core executable for a fixed weight set."""

    def __init__(self, nc):
        import jax
        import concourse.mybir as mybir
        from jax.sharding import Mesh, PartitionSpec
        from jax.experimental.shard_map import shard_map
        from concourse import bass2jax
        from concourse.bass2jax import _bass_exec_p, partition_id_tensor

        bass2jax.install_neuronx_cc_hook()
        self.nc = nc
        partition_name = (nc.partition_id_tensor.name
                          if nc.partition_id_tensor else None)
        in_names, out_names, out_avals, zero_outs = [], [], [], []
        for alloc in nc.m.functions[0].allocations:
            if not isinstance(alloc, mybir.MemoryLocationSet):
                continue
            if alloc.kind == "ExternalInput":
                name = alloc.memorylocations[0].name
                if name != partition_name:
                    in_names.append(name)
            elif alloc.kind == "ExternalOutput":
                shape = tuple(alloc.tensor_shape)
                dtype = mybir.dt.np(alloc.dtype)
                out_names.append(alloc.memorylocations[0].name)
                out_avals.append(jax.core.ShapedArray(shape, dtype))
                zero_outs.append(np.zeros(shape, dtype))
        assert in_names == ["xin"], in_names
        self.out_names = out_names
        self.out_avals = out_avals
        all_in_names = in_names + out_names + (
            [partition_name] if partition_name else [])

        def _body(*args):
            operands = list(args)
            if partition_name is not None:
                operands.append(partition_id_tensor())
            return tuple(_bass_exec_p.bind(
                *operands, out_avals=tuple(out_avals),
                in_names=tuple(all_in_names), out_names=tuple(out_names),
                lowering_input_output_aliases=(), sim_require_finite=True,
                sim_require_nnan=True, nc=nc))

        devices = jax.devices()[:NCORES]
        mesh = Mesh(np.asarray(devices), ("core",))
        nin = len(in_names) + len(out_avals)
        self.sharded = jax.jit(
            shard_map(_body, mesh=mesh,
                      in_specs=(PartitionSpec("core"),) * nin,
                      out_specs=(PartitionSpec("core"),) * len(out_avals),
                      check_rep=False),
            keep_unused=True)
        self.zeros = [jax.device_put(
            np.zeros((NCORES * z.shape[0], *z.shape[1:]), z.dtype))
            for z in zero_outs]
        self.jax = jax

    def put(self, xs):
        return self.jax.device_put(np.concatenate(xs, axis=0))

    def dispatch(self, xin_dev):
        return self.sharded(xin_dev, *self.zeros)

    def run(self, xs):
        outs = self.dispatch(self.put(xs))
        o = np.asarray(outs[0])
        return [o.reshape(NCORES, *self.out_avals[0].shape)[c]
                for c in range(NCORES)]


_CACHE = {}


def _weights_key(inputs):
    import hashlib
    hsh = hashlib.blake2b(digest_size=16)
    for n in WEIGHT_NAMES:
        hsh.update(np.ascontiguousarray(np.asarray(inputs[n])).tobytes())
    return hsh.hexdigest()


def get_runner(inputs):
    key = _weights_key(inputs)
    if _CACHE.get("key") != key:
        weights = fold_weights(inputs)
        nc = build_nc(weights)
        _CACHE.update(key=key, runner=_Runner(nc))
    return _CACHE["runner"]


def kernel(**inputs):
    runner = get_runner(inputs)
    outs = runner.run(pack_inputs(inputs))
    return assemble_output(outs)
